# revision 10
# baseline (speedup 1.0000x reference)
"""Pipeline X2: banded-matmul depthwise + DMA-xbar transpose + pointwise, bf16.

All free-dim walks are contiguous-inner (the PE moving operand and ACT/DVE
copies need >=16B inner runs; strided 2B walks run ~5x slower).

Per core (4 batches):
  x host-prepped to [b, h, 114w, 96ci] bf16 (W zero-padded, ci INNER)
  DW:  per 8-j tile (2 PSUM banks, 4 j each): 3 taps x 2 sub-blocks,
       lhsT = band B_v [112h, 128i] (i-padded for FWL),
       rhs = xt[:, j0+4t+v : +4, :] [112h, (4j, 96ci)] N=384
       -> PSUM qp [128i, 8j, 128cpad] (each 4j-slice = one bank)
       evac -> qsb [112 i, 112 j, 128 cpad]   (contiguous both sides)
  TR:  one DMA xbar transpose per batch (off the PE): the HW xbar folds the
       flat free dim by 128, so qsb's inner dim is ci padded to 128:
       qsb [112, (j,c128)] -> qtb [128 ci, 112 j, 112 i] (parts 96..127 junk)
  PW:  per 4-j chunk: rhs = qtb[0:96, j0:j0+4, :] (native layout, i inner),
       co split {128, 64}: 2 matmuls N=448 -> ys staging [co, j, i]
  y stored [b, co, j, i]; host transposes to [b, co, i, j] and upcasts fp32.

Emission order DW(b), xbar(b), PW(b-1) keeps the PE fed while the xbar
transpose of batch b is in flight.
"""

import numpy as np
import ml_dtypes

from concourse import bacc, mybir
from concourse import tile
from concourse.bass_utils import run_bass_kernel_spmd

F32 = mybir.dt.float32
BF16 = mybir.dt.bfloat16

B, C_IN, C_OUT, H, W = 32, 96, 192, 112, 112
N_CORES = 8
B_PER = B // N_CORES
WP = W + 2                      # 114 padded width
IPAD = 128                      # band i-dim padded for FWL weight loads
CPAD = 128                      # qsb ci padded to the xbar fold width
JB = 4                          # j rows per DW matmul (N = 4*96 = 384)
JT = 8                          # j rows per DW PSUM tile (2 banks)
N_JT = W // JT                  # 14 DW tiles per batch
RPC = 4                         # j rows per PW chunk -> N = 448
N_CHUNKS = W // RPC             # 28
CPB = 7                         # PW chunks per out-DMA block (28 j rows)
N_BLOCKS = N_CHUNKS // CPB      # 4

_NC = None
LAST_RESULTS = None


def _build():
    nc = bacc.Bacc("TRN2", target_bir_lowering=False, debug=False,
                   num_devices=N_CORES)

    # x: [b, h, wp, ci]  (host pre-transposed + W-padded, ci inner, bf16)
    x_d = nc.dram_tensor("x", [B_PER, H, WP, C_IN], BF16, kind="ExternalInput")
    band_d = nc.dram_tensor("band", [H, 3, IPAD], BF16, kind="ExternalInput")
    wpcT_d = nc.dram_tensor("wpcT", [C_IN, C_OUT], BF16, kind="ExternalInput")
    # y stored transposed: [b, co, j, i]; host swaps back to [b, co, i, j]
    y_d = nc.dram_tensor("y", [B_PER, C_OUT, W, H], BF16,
                         kind="ExternalOutput")

    with tile.TileContext(nc) as tc:
        with (
            tc.tile_pool(name="consts", bufs=1) as consts,
            tc.tile_pool(name="xin", bufs=2) as xin,
            tc.tile_pool(name="qsb", bufs=2) as qsbp,
            tc.tile_pool(name="qtb", bufs=2) as qtbp,
            tc.tile_pool(name="ysa", bufs=2) as ysap,
            tc.tile_pool(name="ysb", bufs=2) as ysbp,
            tc.tile_pool(name="qp", bufs=3, space="PSUM") as qpp,
            tc.tile_pool(name="ypa", bufs=3, space="PSUM") as ypap,
            tc.tile_pool(name="ypb", bufs=2, space="PSUM") as ypbp,
        ):
            band_sb = consts.tile([H, 3, IPAD], BF16)
            nc.sync.dma_start(band_sb[:], band_d[:])
            wpc_sb = consts.tile([C_IN, C_OUT], BF16)
            nc.sync.dma_start(wpc_sb[:], wpcT_d[:])

            copy_ctr = 0
            qtbs = [None] * B_PER

            def emit_dw(b):
                nonlocal copy_ctr
                xt = xin.tile([H, WP, C_IN], BF16, name=f"xt{b}", tag="xt")
                nc.scalar.dma_start(xt[:], x_d[b])

                # qsb [112 i, 112 j, 128 cpad] (ci inner)
                qsb = qsbp.tile([H, W, CPAD], BF16, name=f"qsb{b}", tag="qsb")
                # zero the pad cols (the xbar reads them; overlaps DW compute
                # on the otherwise-idle gpsimd engine)
                nc.gpsimd.memset(qsb[:, :, C_IN:CPAD], 0)
                qtb = qtbp.tile([CPAD, W, H], BF16, name=f"qtb{b}", tag="qtb")
                HJ = W // 2
                for jt in range(W // JB):
                    # one PSUM bank per 4-j block, contiguous [128, 384] out
                    qp = qpp.tile([IPAD, 512], F32, name="qp", tag="qp")
                    j0 = jt * JB
                    for v in range(3):
                        nc.tensor.matmul(
                            qp[:, 0:JB * C_IN],
                            band_sb[:, v, :],
                            xt[:, j0 + v:j0 + v + JB, :],
                            start=(v == 0), stop=(v == 2),
                        )
                    src = qp[0:H, 0:JB * C_IN]
                    dst = qsb[:, j0:j0 + JB, 0:C_IN]
                    if copy_ctr % 2 == 0:
                        nc.scalar.copy(dst, src)
                    else:
                        nc.vector.tensor_copy(dst, src)
                    copy_ctr += 1
                    # DMA xbar transpose in j-halves (all 16 DMA engines,
                    # off the PE), launched as soon as each half is evac'd:
                    # [112 i, (j, c128)] -> [128 c, j, 112 i]
                    if j0 + JB == HJ:
                        nc.sync.dma_start(
                            qtb[:, 0:HJ, :],
                            qsb[:, 0:HJ, :].rearrange("i j c -> i (j c)"),
                            transpose=True)
                    elif j0 + JB == W:
                        nc.sync.dma_start(
                            qtb[:, HJ:W, :],
                            qsb[:, HJ:W, :].rearrange("i j c -> i (j c)"),
                            transpose=True)
                qtbs[b] = qtb

            def emit_pw(b):
                nonlocal copy_ctr
                qtb = qtbs[b]
                for q in range(N_BLOCKS):
                    ysa = ysap.tile([128, CPB * RPC, H], BF16,
                                    name=f"ysa{b}_{q}", tag="ysa")
                    ysb = ysbp.tile([64, CPB * RPC, H], BF16,
                                    name=f"ysb{b}_{q}", tag="ysb")
                    for t in range(CPB):
                        j0 = (q * CPB + t) * RPC
                        rhs = qtb[0:C_IN, j0:j0 + RPC, :]
                        ya = ypap.tile([128, RPC, H], F32, name="ya", tag="ya")
                        nc.tensor.matmul(ya[:], wpc_sb[:, 0:128], rhs,
                                         start=True, stop=True)
                        yb = ypbp.tile([64, RPC, H], F32, name="yb", tag="yb")
                        nc.tensor.matmul(yb[:], wpc_sb[:, 128:192], rhs,
                                         start=True, stop=True)
                        for ys, yp in ((ysa, ya), (ysb, yb)):
                            dst = ys[:, RPC * t:RPC * (t + 1), :]
                            if copy_ctr % 2 == 0:
                                nc.scalar.copy(dst, yp[:])
                            else:
                                nc.vector.tensor_copy(dst, yp[:])
                            copy_ctr += 1
                    r0 = q * CPB * RPC
                    nc.sync.dma_start(
                        y_d[b, 0:128, r0:r0 + CPB * RPC, :], ysa[:])
                    nc.sync.dma_start(
                        y_d[b, 128:192, r0:r0 + CPB * RPC, :], ysb[:])

            for b in range(B_PER):
                emit_dw(b)
                if b > 0:
                    emit_pw(b - 1)
            emit_pw(B_PER - 1)

    nc.compile()
    return nc


def _prep_inputs(x, w_pc, w_dc):
    x = np.asarray(x, dtype=np.float32)
    k3 = np.asarray(w_dc, dtype=np.float32).reshape(3, 3)
    Wm = np.asarray(w_pc, dtype=np.float32).reshape(C_OUT, C_IN)

    # [b, h, 114 w, ci]: transpose + W-pad, ci inner, bf16
    xp = np.zeros((B, H, WP, C_IN), dtype=np.float32)
    xp[:, :, 1:1 + W, :] = x.transpose(0, 2, 3, 1)
    xp = xp.astype(ml_dtypes.bfloat16)

    # band[h, v, i] = k3[h - i + 1, v], i-padded to 128
    band = np.zeros((H, 3, IPAD), dtype=np.float32)
    hh, ii = np.meshgrid(np.arange(H), np.arange(H), indexing="ij")
    u = hh - ii + 1
    m = (u >= 0) & (u < 3)
    for v in range(3):
        bv = np.zeros((H, H), dtype=np.float32)
        bv[m] = k3[u[m], v]
        band[:, v, :H] = bv
    band = band.astype(ml_dtypes.bfloat16)

    wpcT = np.ascontiguousarray(Wm.T).astype(ml_dtypes.bfloat16)
    return xp, band, wpcT


def kernel(x, w_pc, w_dc, _trace=False):
    global _NC, LAST_RESULTS
    if _NC is None:
        _NC = _build()

    xp, band, wpcT = _prep_inputs(x, w_pc, w_dc)
    in_maps = [
        {"x": np.ascontiguousarray(xp[i * B_PER:(i + 1) * B_PER]),
         "band": band, "wpcT": wpcT}
        for i in range(N_CORES)
    ]
    res = run_bass_kernel_spmd(_NC, in_maps, list(range(N_CORES)),
                               trace=_trace)
    LAST_RESULTS = res
    # y stored [b, co, j, i] on device -> [b, co, i, j]
    y = np.concatenate([res.results[i]["y"] for i in range(N_CORES)], axis=0)
    y = np.asarray(y, dtype=np.float32).transpose(0, 1, 3, 2)
    return np.ascontiguousarray(y)
